# revision 30
# baseline (speedup 1.0000x reference)
"""Trainium2 Bass kernel for nn_BuildK (27-neighborhood kNN softmax weights).

v2 design, tuned for the axon backend cost model (per-instruction overhead
dominates): minimize instruction count.

- Sort phase (2 z-chunks of 32): 27 neighbor keys packed as
  trunc(|diff|) * (1 + (2d+sign)*2^-23) in a contiguous [32, FS] tile,
  sorted by a batched Batcher odd-even mergesort (each network level's
  comparator groups run as single strided-AP min/max instructions).
  Decode of the 8 nearest runs batched over ranks. Results (neighbor
  values Wd, neighbor dir indices Id) are staged to DRAM scratch.
- Sigma phase: rowwise unbiased variance -> logit scale.
- Dot phase (2 z-chunks of 32): pairwise feature distances via the exact
  symmetry dist2(v,d) = dist2(v+off_d, 26-d); 13 extended difference
  planes (eps dropped - well within tolerance), per-dir logits selected
  into rank slots with broadcast index-compare ops, single Exp, softmax.
"""

import sys

sys.path.insert(0, "/opt/trn_rl_repo")

import numpy as np

H, M, N = 64, 128, 128
NCORES = 8
YS = M // NCORES          # 16 owned y rows per core
YE = YS + 2               # 18 = sort region (owned + 1 halo each side)
YI = YS + 4               # 20 = input slab y extent (halo 2)
ZE = H + 2                # 66 = z extent with periodic wrap rows
KN = 9
SZ = 32                   # z chunk
FS = SZ * YE              # 576 free elems per sort row

OFFS = [(oz, oy, ox) for oz in (-1, 0, 1) for oy in (-1, 0, 1)
        for ox in (-1, 0, 1)]            # reference enumeration; 13 = center


# --------------------------------------------------------------------------
# Batched Batcher odd-even mergesort schedule for 32 rows.
# Groups: (base, d1, n1, d2, n2, r) -> compare rows (i, i+r),
# i = base + a*d1 + b*d2.  Groups capped at 8 pairs (scratch size).
# --------------------------------------------------------------------------

def _oddeven_comparators(n):
    ops = []

    def merge(lo, m, r):
        step = r * 2
        if step < m:
            merge(lo, m, step)
            merge(lo + r, m, step)
            for i in range(lo + r, lo + m - r, step):
                ops.append((i, i + r))
        else:
            ops.append((lo, lo + r))

    def srt(lo, m):
        if m > 1:
            h = m // 2
            srt(lo, h)
            srt(lo + h, h)
            merge(lo, m, 1)

    srt(0, n)
    return ops


def _grid_decompose(idxs):
    idxs = sorted(idxs)
    grids = []
    rest = idxs
    while rest:
        if len(rest) == 1:
            grids.append((rest[0], 1, 1, 1, 1))
            break
        d1 = rest[1] - rest[0]
        runs = []
        s = rest[0]
        cnt = 1
        for a, b in zip(rest, rest[1:]):
            if b - a == d1:
                cnt += 1
            else:
                runs.append((s, cnt))
                s = b
                cnt = 1
        runs.append((s, cnt))
        n1 = min(c for (_, c) in runs)
        starts = []
        leftover = []
        for (st, c) in runs:
            starts.append(st)
            if c > n1:
                leftover.extend(range(st + n1 * d1, st + c * d1, d1))
        ok2 = True
        d2 = starts[1] - starts[0] if len(starts) > 1 else 1
        for a, b in zip(starts, starts[1:]):
            if b - a != d2:
                ok2 = False
        if ok2:
            grids.append((starts[0], d1, n1, d2, len(starts)))
            rest = sorted(leftover)
        else:
            st, c = runs[0]
            grids.append((st, d1, c, 1, 1))
            rest = sorted(set(rest) - set(range(st, st + c * d1, d1)))
    return grids


def batched_schedule(n=32, maxpairs=8):
    ops = _oddeven_comparators(n)
    level = [0] * n
    lv = []
    for (i, j) in ops:
        l = max(level[i], level[j])
        lv.append((l, i, j))
        level[i] = l + 1
        level[j] = l + 1
    from collections import defaultdict
    bylvr = defaultdict(list)
    for (l, i, j) in lv:
        bylvr[(l, j - i)].append(i)
    sched = []
    for (l, r) in sorted(bylvr.keys()):
        for (base, d1, n1, d2, n2) in _grid_decompose(bylvr[(l, r)]):
            # split so n1*n2 <= maxpairs (split the bigger factor)
            cells = [(base + b * d2, d1, n1) for b in range(n2)]
            cur = []
            cnt = 0
            for (b0, dd1, nn1) in cells:
                while nn1 > 0:
                    take = min(nn1, maxpairs - cnt)
                    cur.append((b0, dd1, take))
                    b0 += take * dd1
                    nn1 -= take
                    cnt += take
                    if cnt == maxpairs:
                        sched.append((cur, r))
                        cur = []
                        cnt = 0
            if cur:
                sched.append((cur, r))
    # each entry: (list of (base, d1, n1) runs, r); regroup runs into
    # (base, d1, n1, d2, n2) when runs are evenly spaced with equal n1
    out = []
    for (runs, r) in sched:
        if len(runs) == 1:
            b, d1, n1 = runs[0]
            out.append((b, d1, n1, 1, 1, r))
        else:
            n1s = set(x[2] for x in runs)
            d1s = set(x[1] for x in runs)
            bs = [x[0] for x in runs]
            gaps = set(b2 - b1 for b1, b2 in zip(bs, bs[1:]))
            if len(n1s) == 1 and len(d1s) == 1 and len(gaps) == 1:
                out.append((bs[0], d1s.pop(), n1s.pop(), gaps.pop(),
                            len(bs), r))
            else:
                for (b, d1, n1) in runs:
                    out.append((b, d1, n1, 1, 1, r))
    return out


SORT_SCHED = batched_schedule(32, maxpairs=16)


# --------------------------------------------------------------------------
# Bass graph
# --------------------------------------------------------------------------

def build_bass(ks_value: float, reps: int = 1):
    from concourse import bacc, mybir
    from concourse import tile
    from concourse.alu_op_type import AluOpType as op
    from concourse.bass_types import AP

    f32 = mybir.dt.float32
    AF = mybir.ActivationFunctionType

    nc = bacc.Bacc("TRN2", target_bir_lowering=False, debug=False,
                   num_devices=NCORES)

    xin = nc.dram_tensor("xin", [128, 3, ZE, YI], f32,
                         kind="ExternalInput").ap()
    cst = nc.dram_tensor("cst", [128, 27, 1], f32, kind="ExternalInput").ap()
    outd = nc.dram_tensor("out", [128, KN, H, YS], f32,
                          kind="ExternalOutput").ap()
    Wd = nc.dram_tensor("Wd", [128, KN, ZE, YE], f32, kind="Internal").ap()
    Id = nc.dram_tensor("Id", [128, 8, H, YS], f32, kind="Internal").ap()

    dve = nc.vector
    act = nc.scalar

    def bcast0(ap, n):
        """Insert a stride-0 dim of length n right after the partition dim."""
        pairs = [list(p) for p in ap.ap]
        newpairs = [pairs[0], [0, n]] + pairs[1:]
        return AP(ap.tensor, ap.offset, newpairs)

    def rows_ap(tile_handle_ap, R, F, base, d1, n1, d2, n2, foff=0, flen=None):
        """AP selecting rows {base + a*d1 + b*d2} of a [128, R, F] tile."""
        if flen is None:
            flen = F
        pairs = [[R * F, 128]]
        if n2 > 1:
            pairs.append([d2 * F, n2])
        if n1 > 1:
            pairs.append([d1 * F, n1])
        pairs.append([1, flen])
        return AP(tile_handle_ap.tensor, tile_handle_ap.offset + base * F + foff,
                  pairs)

    with tile.TileContext(nc) as tc:
      for _rep in range(reps):
        with tc.tile_pool(name="pp", bufs=1) as pp:
            scalem = pp.tile([128, H, YS], f32, tag="scalem")
            cstt = pp.tile([128, 27, 1], f32, tag="cstt")
            nc.sync.dma_start(out=cstt[:], in_=cst[:])

            # ================= sort phase =================
            for zc in (0, SZ):
                with tc.tile_pool(name="sortp", bufs=1) as sp:
                    X3c = sp.tile([128, 3, SZ + 2, YI], f32, tag="X3c")
                    nc.sync.dma_start(out=X3c[:],
                                      in_=xin[:, :, zc:zc + SZ + 2, :])
                    K = sp.tile([128, 32, FS], f32, tag="K")
                    T2 = sp.tile([128, 14, FS], f32, tag="T2")
                    T3 = sp.tile([128, 14, FS], f32, tag="T3")
                    SCR = sp.tile([128, 16, FS], f32, tag="SCR")
                    Wstage = sp.tile([128, KN, SZ, YE], f32, tag="Wstage")

                    cvw = X3c[:, 1, 1:1 + SZ, 1:1 + YE]     # [32,18]

                    dve.memset(K[:, 27:32], 3.0e38)

                    # ---- prep: keys for all 27 dirs ----
                    # grouped diffs straight into K rows, then a 2-pack
                    # ALU pipeline transforms them into packed sort keys
                    for g in range(9):
                        oz, oy = g // 3 - 1, g % 3 - 1
                        vv = X3c[:, :, 1 + oz:1 + oz + SZ,
                                 1 + oy:1 + oy + YE]
                        dve.tensor_tensor(out=K[:, g * 3:g * 3 + 3],
                                          in0=vv, in1=bcast0(cvw, 3),
                                          op=op.subtract)
                    for (r0, nr) in ((0, 14), (14, 13)):
                        kp = K[:, r0:r0 + nr]
                        t1 = SCR[:, 0:nr]
                        t2 = T2[:, 0:nr]
                        t3 = T3[:, 0:nr]
                        dve.scalar_tensor_tensor(out=t2, in0=kp,
                                                 scalar=-1.0, in1=kp,
                                                 op0=op.mult, op1=op.max)
                        dve.tensor_scalar(out=t3, in0=t2, scalar1=257.0,
                                          scalar2=None, op0=op.mult)
                        dve.tensor_tensor(out=t2, in0=t3, in1=t2,
                                          op=op.subtract)
                        dve.tensor_tensor(out=t3, in0=t3, in1=t2,
                                          op=op.subtract)       # t3 = hi
                        dve.tensor_scalar(out=t1, in0=kp, scalar1=0.0,
                                          scalar2=None, op0=op.is_gt)
                        cr = cstt[:, r0:r0 + nr, :]
                        dve.scalar_tensor_tensor(
                            out=t1, in0=t1,
                            scalar=float(np.float32(2.0**-23)),
                            in1=cr.broadcast_to((128, nr, FS)),
                            op0=op.mult, op1=op.add)            # t1 = m
                        dve.tensor_tensor(out=kp, in0=t3, in1=t1,
                                          op=op.mult)

                    # ---- batched Batcher sort ----
                    for (base, d1, n1, d2, n2, r) in SORT_SCHED:
                        npairs = n1 * n2
                        lo = rows_ap(K[:], 32, FS, base, d1, n1, d2, n2)
                        hi = rows_ap(K[:], 32, FS, base + r, d1, n1, d2, n2)
                        sc = SCR[:, 0:npairs]
                        dve.tensor_tensor(out=sc, in0=lo, in1=hi, op=op.min)
                        dve.tensor_tensor(out=hi, in0=lo, in1=hi, op=op.max)
                        dve.tensor_copy(out=lo, in_=sc)

                    # ---- batched decode of ranks 1..8 ----
                    KS = K[:, 1:9]
                    e1 = SCR[:, 0:8]
                    e2 = T2[:, 0:8]
                    e3 = T3[:, 0:8]
                    sc = SCR[:, 8:16]
                    C = float(1.5 * 2.0**23)
                    dve.tensor_scalar(out=e1, in0=KS, scalar1=257.0,
                                      scalar2=None, op0=op.mult)
                    dve.tensor_tensor(out=e2, in0=e1, in1=KS, op=op.subtract)
                    dve.tensor_tensor(out=e3, in0=e1, in1=e2, op=op.subtract)
                    dve.tensor_tensor(out=e1, in0=KS, in1=e3, op=op.subtract)
                    dve.tensor_scalar(out=e2, in0=e3, scalar1=1e-30,
                                      scalar2=None, op0=op.add)
                    dve.reciprocal(out=sc, in_=e2)
                    dve.tensor_tensor(out=e1, in0=e1, in1=sc, op=op.mult)
                    dve.tensor_scalar(out=e1, in0=e1, scalar1=float(2.0**23),
                                      scalar2=C, op0=op.mult, op1=op.add)
                    dve.tensor_scalar(out=e2, in0=e1, scalar1=C, scalar2=None,
                                      op0=op.subtract)          # code
                    dve.tensor_scalar(out=e1, in0=e2, scalar1=-0.5,
                                      scalar2=0.5, op0=op.add, op1=op.mult)
                    dve.tensor_scalar(out=e1, in0=e1, scalar1=C, scalar2=None,
                                      op0=op.add)
                    dve.tensor_scalar(out=e1, in0=e1, scalar1=C, scalar2=None,
                                      op0=op.subtract)          # e1 = delta
                    # idx -> contiguous staging in SCR, then DRAM
                    d_view = AP(SCR[:].tensor, SCR[:].offset + 1,
                                [[16 * FS, 128], [FS, 8], [YE, SZ], [1, YS]])
                    idq = AP(SCR[:].tensor, SCR[:].offset + 8 * FS,
                             [[16 * FS, 128], [SZ * YS, 8], [1, SZ * YS]])
                    dve.tensor_copy(out=idq, in_=d_view)
                    nc.sync.dma_start(out=Id[:, :, zc:zc + SZ, :], in_=idq)
                    # sign & neighbor values
                    dve.scalar_tensor_tensor(out=sc, in0=e1, scalar=-2.0,
                                             in1=e2, op0=op.mult, op1=op.add)
                    dve.tensor_scalar(out=sc, in0=sc, scalar1=2.0,
                                      scalar2=-1.0, op0=op.mult, op1=op.add)
                    dve.tensor_tensor(out=sc, in0=sc, in1=e3, op=op.mult)
                    dve.tensor_tensor(out=Wstage[:, 1:9], in0=sc,
                                      in1=bcast0(cvw, 8), op=op.add)
                    dve.tensor_copy(out=Wstage[:, 0], in_=cvw)
                    nc.sync.dma_start(out=Wd[:, :, 1 + zc:1 + zc + SZ, :],
                                      in_=Wstage[:])
                    # z wrap rows of Wd, straight from the staging tile
                    if zc == 0:
                        nc.sync.dma_start(out=Wd[:, :, ZE - 1:ZE, :],
                                          in_=Wstage[:, :, 0:1, :])
                    else:
                        nc.sync.dma_start(out=Wd[:, :, 0:1, :],
                                          in_=Wstage[:, :, SZ - 1:SZ, :])

            # ================= sigma phase =================
            with tc.tile_pool(name="sigp", bufs=1) as sg:
                Wf = sg.tile([128, KN, ZE, YE], f32, tag="Wf")
                nc.sync.dma_start(out=Wf[:], in_=Wd[:])
                Wo = Wf[:, :, 1:1 + H, 1:1 + YS]      # [9,64,16]
                sq9 = sg.tile([128, KN, H * YS], f32, tag="sq9")
                S1 = sg.tile([128, H, YS], f32, tag="S1")
                S2 = sg.tile([128, H, YS], f32, tag="S2")
                tv = sg.tile([128, H, YS], f32, tag="tv")
                wfa = Wf[:]
                wro = AP(wfa.tensor, wfa.offset + YE + 1,
                         [[KN * ZE * YE, 128], [YE, H], [1, YS],
                          [ZE * YE, KN]])
                dve.tensor_reduce(out=S1[:], in_=wro,
                                  axis=mybir.AxisListType.X, op=op.add)
                dve.tensor_tensor(out=sq9[:], in0=Wo, in1=Wo, op=op.mult)
                sqa = sq9[:]
                sqro = AP(sqa.tensor, sqa.offset,
                          [[KN * H * YS, 128], [1, H * YS], [H * YS, KN]])
                dve.tensor_reduce(out=S2[:], in_=sqro,
                                  axis=mybir.AxisListType.X, op=op.add)
                dve.tensor_tensor(out=tv[:], in0=S1[:], in1=S1[:], op=op.mult)
                dve.scalar_tensor_tensor(out=tv[:], in0=tv[:],
                                         scalar=-1.0 / 9.0, in1=S2[:],
                                         op0=op.mult, op1=op.add)  # tvar
                dve.tensor_scalar(out=S1[:], in0=tv[:], scalar1=0.0,
                                  scalar2=None, op0=op.is_equal)
                dve.tensor_tensor(out=S1[:], in0=S1[:], in1=tv[:], op=op.add)
                dve.reciprocal(out=S2[:], in_=S1[:])
                dve.tensor_scalar(out=S2[:], in0=S2[:],
                                  scalar1=-4.0 / (ks_value * ks_value),
                                  scalar2=None, op0=op.mult)
                dve.tensor_scalar(out=S1[:], in0=tv[:], scalar1=0.0,
                                  scalar2=None, op0=op.not_equal)
                dve.tensor_tensor(out=scalem[:], in0=S2[:], in1=S1[:],
                                  op=op.mult)

            # ================= dot phase =================
            DXM = [0, 3, 6, 9, 12]     # ox == -1
            DX0 = [1, 4, 7, 10]        # ox == 0
            DXP = [2, 5, 8, 11]        # ox == +1
            PL = (SZ + 1) * 17         # 561 elems per distance plane
            for zc in (0, SZ):
                with tc.tile_pool(name="dotp", bufs=1) as dp:
                    Wm = dp.tile([128, KN, SZ + 2, YE], f32, tag="Wm")
                    nc.sync.dma_start(out=Wm[:],
                                      in_=Wd[:, :, zc:zc + SZ + 2, :])
                    wrb = dp.tile([128, KN, SZ + 2, YE], f32, tag="wrb")
                    idxc = dp.tile([128, 8, SZ * YS], f32, tag="idxc")
                    nc.sync.dma_start(out=idxc[:],
                                      in_=Id[:, :, zc:zc + SZ, :])
                    diff = dp.tile([128, KN, SZ + 1, 17], f32, tag="diff")
                    # ox != 0 cats hold [plane, rotated-plane] pairs
                    drxm = dp.tile([128, 5, 2, PL], f32, tag="drxm")
                    drx0 = dp.tile([128, 4, PL], f32, tag="drx0")
                    drxp = dp.tile([128, 4, 2, PL], f32, tag="drxp")
                    L2 = dp.tile([128, 2, SZ * YS], f32, tag="L2")
                    es8 = dp.tile([128, 8, SZ * YS], f32, tag="es8")
                    oc8 = dp.tile([128, 8, SZ * YS], f32, tag="oc8")
                    ee9 = dp.tile([128, KN, SZ * YS], f32, tag="ee9")
                    s4 = dp.tile([128, 4, SZ * YS], f32, tag="s4")

                    CAT = {}
                    for i, d in enumerate(DXM):
                        CAT[d] = (drxm, True, i)
                    for i, d in enumerate(DX0):
                        CAT[d] = (drx0, False, i)
                    for i, d in enumerate(DXP):
                        CAT[d] = (drxp, True, i)

                    # 13 extended squared-distance planes, grouped by ox so
                    # one rotated slab buffer suffices
                    def do_dist(d, wbt):
                        oz, oy, ox = OFFS[d]
                        y0 = -1 if oy == 1 else 0
                        a = Wm[:, :, 1:2 + SZ, 1 + y0:1 + y0 + 17]
                        b = wbt[:, :, 1 + oz:1 + oz + SZ + 1,
                                1 + y0 + oy:1 + y0 + oy + 17]
                        dve.tensor_tensor(out=diff[:], in0=a, in1=b,
                                          op=op.subtract)
                        dve.tensor_tensor(out=diff[:], in0=diff[:],
                                          in1=diff[:], op=op.mult)
                        dv = AP(diff[:].tensor, diff[:].offset,
                                [[KN * PL, 128],
                                 [17, SZ + 1], [1, 17],
                                 [PL, KN]])
                        cat, paired, k = CAT[d]
                        dst = cat[:, k, 0] if paired else cat[:, k]
                        dve.tensor_reduce(out=dst, in_=dv,
                                          axis=mybir.AxisListType.X,
                                          op=op.add)

                    # ox == -1 dirs need W at x-1 (wr0-style rotation)
                    nc.sync.dma_start(out=wrb[1:128],
                                      in_=Wd[0:127, :, zc:zc + SZ + 2, :])
                    nc.sync.dma_start(out=wrb[0:1],
                                      in_=Wd[127:128, :, zc:zc + SZ + 2, :])
                    for d in DXM:
                        do_dist(d, wrb)
                    for d in DX0:
                        do_dist(d, Wm)
                    # ox == +1 dirs need W at x+1 (wr2-style rotation)
                    nc.sync.dma_start(out=wrb[0:127],
                                      in_=Wd[1:128, :, zc:zc + SZ + 2, :])
                    nc.sync.dma_start(out=wrb[127:128],
                                      in_=Wd[0:1, :, zc:zc + SZ + 2, :])
                    for d in DXP:
                        do_dist(d, wrb)

                    # rotate the ox != 0 plane groups across partitions into
                    # the adjacent slot of each [plane, rot] pair
                    nc.sync.dma_start(out=drxm[0:127, :, 1],
                                      in_=drxm[1:128, :, 0])
                    nc.sync.dma_start(out=drxm[127:128, :, 1],
                                      in_=drxm[0:1, :, 0])
                    nc.sync.dma_start(out=drxp[1:128, :, 1],
                                      in_=drxp[0:127, :, 0])
                    nc.sync.dma_start(out=drxp[0:1, :, 1],
                                      in_=drxp[127:128, :, 0])

                    # logits for each (d, 26-d) pair in one op, then select
                    scv = scalem[:, zc:zc + SZ, :]

                    first = True
                    for d in range(13):
                        oz, oy, ox = OFFS[d]
                        y0 = -1 if oy == 1 else 0
                        cat, paired, k = CAT[d]
                        base = cat[:]
                        w0 = -y0
                        pstride = (PL if paired else 0) - 17 * oz - oy
                        planesz = 2 * PL if paired else PL
                        nplanes = 5 if cat is drxm else 4
                        pv = AP(base.tensor, base.offset + k * planesz + w0,
                                [[nplanes * planesz, 128],
                                 [pstride, 2], [17, SZ], [1, 16]])
                        dve.tensor_tensor(out=L2[:], in0=pv,
                                          in1=bcast0(scv, 2), op=op.mult)
                        for (sl, dd) in ((0, d), (1, 26 - d)):
                            if first:
                                dve.scalar_tensor_tensor(
                                    out=es8[:], in0=idxc[:],
                                    scalar=float(dd),
                                    in1=bcast0(L2[:, sl], 8),
                                    op0=op.is_equal, op1=op.mult)
                                first = False
                            else:
                                dve.scalar_tensor_tensor(
                                    out=oc8[:], in0=idxc[:],
                                    scalar=float(dd),
                                    in1=bcast0(L2[:, sl], 8),
                                    op0=op.is_equal, op1=op.mult)
                                dve.tensor_tensor(out=es8[:], in0=es8[:],
                                                  in1=oc8[:], op=op.add)

                    # exp, softmax, output
                    dve.memset(ee9[:, 0], 1.0)
                    act.activation(out=ee9[:, 1:9], in_=es8[:], func=AF.Exp)
                    eev = AP(ee9[:].tensor, ee9[:].offset,
                             [[KN * SZ * YS, 128], [1, SZ * YS],
                              [SZ * YS, KN]])
                    dve.tensor_reduce(out=s4[:, 0], in_=eev,
                                      axis=mybir.AxisListType.X, op=op.add)
                    dve.reciprocal(out=s4[:, 1], in_=s4[:, 0])
                    dve.tensor_tensor(out=ee9[:], in0=ee9[:],
                                      in1=bcast0(s4[:, 1], KN), op=op.mult)
                    nc.sync.dma_start(out=outd[:, :, zc:zc + SZ, :],
                                      in_=ee9[:])

    nc.compile()
    return nc


# --------------------------------------------------------------------------
# Host side
# --------------------------------------------------------------------------

_CACHED = {}


def _get_nc(ks_value):
    key = float(ks_value)
    if key not in _CACHED:
        _CACHED[key] = build_bass(key)
    return _CACHED[key]


def _shard_inputs(x):
    """x: [H, M, N] f32 -> list of per-core input maps."""
    maps = []
    zext = np.arange(-1, H + 1) % H
    xs = np.arange(N)
    cstv = np.tile((1.0 + 2.0 * np.arange(27) * 2.0**-23)
                   .astype(np.float32).reshape(1, 27, 1), (128, 1, 1))
    for c in range(NCORES):
        ys = (np.arange(YS * c - 2, YS * c + YS + 2)) % M
        slab = x[zext][:, ys, :]                       # [66, 20, 128]
        a = np.empty((128, 3, ZE, YI), dtype=np.float32)
        for r in range(3):
            xrot = (xs + r - 1) % N
            a[:, r] = slab[:, :, xrot].transpose(2, 0, 1)
        maps.append({"xin": np.ascontiguousarray(a), "cst": cstv})
    return maps


def kernel(input, ksigma, k, w):
    from concourse.bass_utils import run_bass_kernel_spmd

    x = np.asarray(input, dtype=np.float32)
    assert x.shape == (H, M, N)
    ks = float(np.asarray(ksigma).reshape(-1)[0])
    assert int(k) == KN and int(w) == 3

    nc = _get_nc(ks)
    in_maps = _shard_inputs(x)
    res = run_bass_kernel_spmd(nc, in_maps, core_ids=list(range(NCORES)))
    full = np.empty((H, M, N, KN), dtype=np.float32)
    for c in range(NCORES):
        oc = res.results[c]["out"]          # [128, KN, H, YS]
        full[:, YS * c:YS * c + YS] = oc.transpose(2, 3, 0, 1)
    return full.reshape(H * M * N, KN)
